# revision 1
# baseline (speedup 1.0000x reference)
"""GCN (3x GCNConv + mean-pool + linear + sigmoid) on 8 Trainium2 NeuronCores.

Strategy (1D graph partition):
  - Self-loops folded into the edge list (same d^-1/2 normalization).
  - Nodes striped into 8 contiguous shards, padded to a multiple of 128 rows.
  - Edges sharded by TARGET shard; per 128-target block, messages are
    gathered from a replicated node-major feature table (HBM fp32) with
    dma_gather and segment-summed on the PE: psum[f, t] += M[e, f]^T S[e, t],
    with one-hot S generated on-device (is_equal vs iota, 8 chunks/op).
  - Per-layer table (X * deg^-1/2) rebuilt via AllGather of local shards.
  - Readout: one-hot pool matmul partials, AllReduce, divide by counts,
    final matvec + sigmoid.
"""

import numpy as np

import concourse.bass as bass
import concourse.bacc as bacc
import concourse.mybir as mybir
from concourse.tile import TileContext
from concourse.bass_utils import run_bass_kernel_spmd

F32 = mybir.dt.float32
I16 = mybir.dt.int16
OP = mybir.AluOpType
NCORES = 8
D = 128
G = 64  # number of graphs
SGRP = 8  # chunks per fused S-gen op


def cdiv(a, b):
    return -(-a // b)


def preprocess(x, edge_index, batch):
    """Host-side graph partitioning / index prep (numpy only)."""
    N = x.shape[0]
    SHARD = cdiv(N, NCORES)
    SHARD_PAD = cdiv(SHARD, 128) * 128
    NB = SHARD_PAD // 128
    TBL = NCORES * SHARD_PAD
    LO = min(32768, TBL)

    row = edge_index[0].astype(np.int64)
    col = edge_index[1].astype(np.int64)
    deg = np.bincount(col, minlength=N).astype(np.float32) + 1.0
    dis = (1.0 / np.sqrt(deg)).astype(np.float32)


    srow = (row // SHARD) * SHARD_PAD + (row % SHARD)  # table row of source
    core = col // SHARD
    tloc = col % SHARD
    blk = tloc // 128
    toff = tloc % 128
    grp = (srow >= LO).astype(np.int64)

    counts = np.zeros((NCORES, NB, 2), np.int64)
    np.add.at(counts, (core, blk, grp), 1)
    CL = cdiv(counts[:, :, 0].max(axis=0), 128)  # [NB] lo chunks per block
    CH = cdiv(counts[:, :, 1].max(axis=0), 128)  # [NB] hi chunks per block
    nlo = CL * 128
    btot = nlo + CH * 128
    boff = np.zeros(NB + 1, np.int64)
    boff[1:] = np.cumsum(btot)
    TOT = int(boff[-1])

    IDX = np.zeros((NCORES, TOT), np.int64)
    TOF = np.full((NCORES, TOT), 255.0, np.float32)

    order = np.lexsort((grp, blk, core))
    c_s, b_s, g_s = core[order], blk[order], grp[order]
    s_s, t_s = srow[order], toff[order]
    key = (c_s * NB + b_s) * 2 + g_s
    starts = np.r_[0, np.flatnonzero(np.diff(key)) + 1]
    run_len = np.diff(np.r_[starts, len(key)])
    pos = np.arange(len(key)) - np.repeat(starts, run_len)
    dest = boff[b_s] + g_s * nlo[b_s] + pos
    IDX[c_s, dest] = s_s - g_s * LO
    TOF[c_s, dest] = t_s

    per_core = []
    for c in range(NCORES):
        lo_n, hi_n = c * SHARD, min((c + 1) * SHARD, N)
        n_real = hi_n - lo_n
        # wrapped int16 idx: idx j of each 16-group at [j%16, j//16],
        # replicated across the 8 Q7 core partition-groups.
        idx16 = IDX[c].reshape(-1, 16).T.astype(np.int16)
        idx_w = np.tile(idx16, (8, 1))
        toff_w = np.ascontiguousarray(TOF[c].reshape(-1, 128).T)

        dis_sh = np.ones(SHARD_PAD, np.float32)
        dis_sh[:n_real] = dis[lo_n:hi_n]
        bat_sh = np.full(SHARD_PAD, 255.0, np.float32)
        bat_sh[:n_real] = batch[lo_n:hi_n].astype(np.float32)
        x_sh = np.zeros((SHARD_PAD, D), np.float32)
        x_sh[:n_real] = x[lo_n:hi_n]
        per_core.append(dict(
            idx=idx_w, toff=toff_w,
            dis=np.ascontiguousarray(dis_sh.reshape(NB, 128).T),
            disb=np.broadcast_to(dis_sh, (128, SHARD_PAD)).copy(),
            bat=np.ascontiguousarray(bat_sh.reshape(NB, 128).T),
            x=x_sh,
        ))

    gcounts = np.bincount(batch.astype(np.int64), minlength=G).astype(np.float32)
    recip = (1.0 / np.maximum(gcounts, 1.0)).astype(np.float32)
    meta = dict(N=N, SHARD=SHARD, SHARD_PAD=SHARD_PAD, NB=NB, TBL=TBL, LO=LO,
                CL=CL, CH=CH, boff=boff, TOT=TOT, recip=recip)
    return meta, per_core


def build_program(meta, ablate=(), iters=1, gmax=0, spkt=False, nq=4,
                  msg_bufs=3, bf16_tbl=0, sgen_ts=0, zt_bufs=2):
    """gmax: max rows per dma_gather (0 = whole (block,group) in one);
    spkt: single_packet flag for dma_gather; nq: SWDGE queues round-robin."""
    NB, TBL, LO = meta["NB"], meta["TBL"], meta["LO"]
    SHARD_PAD = meta["SHARD_PAD"]
    CL, CH, boff = meta["CL"], meta["CH"], meta["boff"]
    TOT = meta["TOT"]
    NCH = TOT // 128
    W16 = TOT // 16
    CLmax = max(1, int(CL.max()))
    CHmax = max(1, int(CH.max()))
    TDT = mybir.dt.bfloat16 if bf16_tbl else F32

    nc = bacc.Bacc(None, target_bir_lowering=False, debug=False,
                   num_swdge_queues=nq)
    x_d = nc.declare_dram_parameter("x", [SHARD_PAD, D], F32, isOutput=False)
    idx_d = nc.declare_dram_parameter("idx", [128, W16], I16, isOutput=False)
    toff_d = nc.declare_dram_parameter("toff", [128, NCH], F32, isOutput=False)
    dis_d = nc.declare_dram_parameter("dis", [128, NB], F32, isOutput=False)
    disb_d = nc.declare_dram_parameter("disb", [128, SHARD_PAD], F32,
                                       isOutput=False)
    bat_d = nc.declare_dram_parameter("bat", [128, NB], F32, isOutput=False)
    iota_d = nc.declare_dram_parameter("iota", [128, 128], F32, isOutput=False)
    idn_d = nc.declare_dram_parameter("idn", [128, 128], F32, isOutput=False)
    w_d = nc.declare_dram_parameter("w", [3, 128, 128], F32, isOutput=False)
    bcol_d = nc.declare_dram_parameter("bcol", [128, 3], F32, isOutput=False)
    wf_d = nc.declare_dram_parameter("wf", [128, 1], F32, isOutput=False)
    aux_d = nc.declare_dram_parameter("aux", [G, 2], F32, isOutput=False)
    out_d = nc.declare_dram_parameter("out", [G, 1], F32, isOutput=True)

    rg = [list(range(NCORES))]
    qn = [0]

    with TileContext(nc) as tc:
        with (
            tc.tile_pool(name="const", bufs=1) as cp,
            tc.tile_pool(name="sb", bufs=2) as sb,
            tc.tile_pool(name="msg", bufs=msg_bufs) as mp,
            tc.tile_pool(name="spool", bufs=3) as spl,
            tc.tile_pool(name="ps", bufs=2, space="PSUM") as ps,
            tc.tile_pool(name="ps1", bufs=1, space="PSUM") as ps1,
            tc.tile_pool(name="dram", bufs=1, space="DRAM") as dp,
        ):
            idx_t = cp.tile([128, W16], I16)
            toff_t = cp.tile([128, NCH], F32)
            iota_t = cp.tile([128, 128], F32)
            idn_t = cp.tile([128, 128], F32)
            dis_t = cp.tile([128, NB], F32)
            disb_t = cp.tile([128, SHARD_PAD], F32)
            bat_t = cp.tile([128, NB], F32)
            w_t = cp.tile([128, 3, 128], F32)
            bcol_t = cp.tile([128, 3], F32)
            wf_t = cp.tile([128, 1], F32)
            aux_t = cp.tile([G, 2], F32)
            idn_bf = cp.tile([128, 128], TDT)

            def gather(out_tile, src, c0, cnt):
                done = 0
                while done < cnt:
                    n = cnt - done if gmax == 0 else min(cnt - done, gmax // 128)
                    nc.gpsimd.dma_gather(
                        out_tile[:, done:done + n, :], src,
                        idx_t[:, (c0 + done) * 8:(c0 + done + n) * 8],
                        n * 128, n * 128, D, single_packet=bool(spkt),
                        queue_num=qn[0] % nq)
                    qn[0] += 1
                    done += n

            for t, d in ((idx_t, idx_d), (toff_t, toff_d), (iota_t, iota_d),
                         (idn_t, idn_d), (dis_t, dis_d), (disb_t, disb_d),
                         (bat_t, bat_d), (bcol_t, bcol_d), (wf_t, wf_d),
                         (aux_t, aux_d)):
                nc.sync.dma_start(out=t[:], in_=d[:])
            for li in range(3):
                nc.sync.dma_start(out=w_t[:, li, :], in_=w_d[li])
            nc.vector.tensor_copy(idn_bf[:], idn_t[:])

            for _it in range(iters):
                ag_in = [dp.tile([SHARD_PAD, D], TDT, tag=f"agin{i}_{_it}",
                                 name=f"agin{i}_{_it}") for i in range(3)]
                ag_out = [dp.tile([TBL, D], TDT, addr_space="Shared",
                                  tag=f"agout{i}_{_it}", name=f"agout{i}_{_it}")
                          for i in range(3)]
                ar_in = dp.tile([G, D], F32, tag=f"arin{_it}", name=f"arin{_it}")
                ar_out = dp.tile([G, D], F32, addr_space="Shared",
                                 tag=f"arout{_it}", name=f"arout{_it}")

                # ---- table 0 = x * dis (local shard) + AllGather ----
                for b in range(NB):
                    xb = sb.tile([128, 128], F32, tag="xb", bufs=3)
                    nc.sync.dma_start(out=xb[:], in_=x_d[b * 128:(b + 1) * 128, :])
                    tb0 = sb.tile([128, 128], TDT, tag="tblblk", bufs=3)
                    nc.vector.tensor_scalar_mul(tb0[:], xb[:], dis_t[:, b:b + 1])
                    nc.sync.dma_start(out=ag_in[0][b * 128:(b + 1) * 128, :],
                                      in_=tb0[:])
                nc.gpsimd.collective_compute(
                    "AllGather", OP.bypass, replica_groups=rg,
                    ins=[ag_in[0].opt()], outs=[ag_out[0].opt()])

                # ---- 3 GCN layers ----
                pp = ps1.tile([G, 128], F32, tag="pp")
                for li in range(3):
                    last = li == 2
                    tbl_dram = ag_out[li]
                    for b in range(NB):
                        lo_c0 = int(boff[b]) // 128
                        ncl, nch = int(CL[b]), int(CH[b])
                        ntot = ncl + nch + 1  # +1 self-loop transpose
                        zt = ps.tile([128, 128], F32, tag="zt", bufs=zt_bufs)
                        groups = []
                        if ncl:
                            mlo = mp.tile([128, CLmax, 128], TDT, tag="mlo")
                            gather(mlo, tbl_dram[0:LO, :], lo_c0, ncl)
                            groups.append((mlo, lo_c0, ncl))
                        if nch:
                            mhi = mp.tile([128, CHmax, 128], TDT, tag="mhi")
                            gather(mhi, tbl_dram[LO:TBL, :], lo_c0 + ncl, nch)
                            groups.append((mhi, lo_c0 + ncl, nch))
                        # self-loop contribution: zt += tbl_block^T via
                        # HWDGE load + transposing matmul (no Q7, no S-gen)
                        slt = sb.tile([128, 128], TDT, tag="slt", bufs=3)
                        nc.sync.dma_start(
                            out=slt[:],
                            in_=ag_in[li][b * 128:(b + 1) * 128, :])
                        if "onlygather" in ablate:
                            continue
                        nc.tensor.matmul(zt[:], slt[:], idn_bf[:],
                                         start=True, stop=False)
                        k = 1
                        for mt, c0, cnt in groups:
                            for c00 in range(0, cnt, SGRP):
                                gn = min(SGRP, cnt - c00)
                                s8 = spl.tile([128, SGRP, 128], TDT, tag="s8")
                                cid = c0 + c00
                                if sgen_ts:
                                    for c in range(gn):
                                        nc.vector.tensor_scalar(
                                            s8[:, c, :], iota_t[:],
                                            toff_t[:, cid + c:cid + c + 1],
                                            None, OP.is_equal)
                                else:
                                    nc.vector.tensor_tensor(
                                        s8[:, :gn, :],
                                        iota_t[:].unsqueeze(1).broadcast_to(
                                            (128, gn, 128)),
                                        toff_t[:, cid:cid + gn].unsqueeze(2)
                                        .broadcast_to((128, gn, 128)),
                                        OP.is_equal)
                                for c in range(gn):
                                    nc.tensor.matmul(
                                        zt[:], mt[:, c00 + c, :], s8[:, c, :],
                                        start=False, stop=(k == ntot - 1))
                                    k += 1
                        # epilogue (transposed): yT = zT*dis ; ht = W @ yT ;
                        # xT = relu(ht + b) ; xp = xT^T ; table = xp * dis
                        yt = sb.tile([128, 128], F32, tag="yt")
                        nc.vector.tensor_mul(
                            yt[:], zt[:], disb_t[:, b * 128:(b + 1) * 128])
                        ht = ps.tile([128, 128], F32, tag="ht")
                        nc.tensor.matmul(ht[:], w_t[:, li, :], yt[:],
                                         start=True, stop=True)
                        xt = sb.tile([128, 128], F32, tag="xt")
                        nc.scalar.activation(xt[:], ht[:],
                                             mybir.ActivationFunctionType.Relu,
                                             bias=bcol_t[:, li:li + 1])
                        xp = ps.tile([128, 128], F32, tag="xp")
                        nc.tensor.transpose(xp[:], xt[:], idn_t[:])
                        if not last:
                            tb = sb.tile([128, 128], TDT, tag="tblblk", bufs=3)
                            nc.vector.tensor_scalar_mul(tb[:], xp[:],
                                                        dis_t[:, b:b + 1])
                            nc.sync.dma_start(
                                out=ag_in[li + 1][b * 128:(b + 1) * 128, :],
                                in_=tb[:])
                        else:
                            xs = sb.tile([128, 128], F32, tag="xs")
                            nc.vector.tensor_copy(xs[:], xp[:])
                            sp = spl.tile([128, G], F32, tag="sp", bufs=2)
                            nc.vector.tensor_scalar(
                                sp[:], iota_t[:, :G], bat_t[:, b:b + 1], None,
                                OP.is_equal)
                            nc.tensor.matmul(pp[:], sp[:], xs[:],
                                             start=(b == 0), stop=(b == NB - 1))
                    if not last:
                        nc.gpsimd.collective_compute(
                            "AllGather", OP.bypass, replica_groups=rg,
                            ins=[ag_in[li + 1].opt()],
                            outs=[ag_out[li + 1].opt()])

                # ---- readout ----
                psb = sb.tile([G, 128], F32, tag="psb")
                if "onlygather" in ablate:
                    nc.vector.memset(psb[:], 0.0)
                else:
                    nc.vector.tensor_copy(psb[:], pp[:])
                nc.sync.dma_start(out=ar_in[:], in_=psb[:])
                nc.gpsimd.collective_compute(
                    "AllReduce", OP.add, replica_groups=rg,
                    ins=[ar_in.opt()], outs=[ar_out.opt()])
                p2 = sb.tile([G, 128], F32, tag="p2")
                nc.sync.dma_start(out=p2[:], in_=ar_out[:])
                nc.vector.tensor_scalar_mul(p2[:], p2[:], aux_t[:, 0:1])
                pt = ps.tile([128, G], F32, tag="zt")
                nc.tensor.transpose(pt[:], p2[:], idn_t[:G, :G])
                pts = sb.tile([128, G], F32, tag="pts")
                nc.vector.tensor_copy(pts[:], pt[:])
                fin = ps.tile([G, 1], F32, tag="ht")
                nc.tensor.matmul(fin[:], pts[:], wf_t[:], start=True, stop=True)
                ob = sb.tile([G, 1], F32, tag="ob")
                nc.scalar.activation(ob[:], fin[:],
                                     mybir.ActivationFunctionType.Sigmoid,
                                     bias=aux_t[:, 1:2])
                nc.sync.dma_start(out=out_d[:], in_=ob[:])

    nc.compile()
    return nc


def make_in_maps(meta, per_core, W1, b1, W2, b2, W3, b3, Wf, bf):
    iota = np.broadcast_to(np.arange(128, dtype=np.float32), (128, 128)).copy()
    idn = np.eye(128, dtype=np.float32)
    w = np.stack([W1, W2, W3]).astype(np.float32)
    bcol = np.stack([b1, b2, b3], axis=1).astype(np.float32)
    aux = np.stack([meta["recip"],
                    np.full(G, float(np.asarray(bf).reshape(-1)[0]), np.float32)],
                   axis=1)
    in_maps = []
    for c in range(NCORES):
        pc = per_core[c]
        in_maps.append(dict(
            x=pc["x"], idx=pc["idx"], toff=pc["toff"], dis=pc["dis"],
            disb=pc["disb"], bat=pc["bat"], iota=iota, idn=idn, w=w, bcol=bcol,
            wf=np.asarray(Wf, np.float32).reshape(128, 1),
            aux=aux,
        ))
    return in_maps


def kernel(x, edge_index, batch, W1, b1, W2, b2, W3, b3, Wf, bf):
    x = np.asarray(x, np.float32)
    edge_index = np.asarray(edge_index)
    batch = np.asarray(batch)
    meta, per_core = preprocess(x, edge_index, batch)
    nc = build_program(meta)
    in_maps = make_in_maps(meta, per_core, W1, b1, W2, b2, W3, b3, Wf, bf)
    res = run_bass_kernel_spmd(nc, in_maps, list(range(NCORES)))
    return np.asarray(res.results[0]["out"], np.float32)



# revision 3
# speedup vs baseline: 2.0510x; 2.0510x over previous
"""GCN (3x GCNConv + mean-pool + linear + sigmoid) on 8 Trainium2 NeuronCores.

v4: hardware-looped (For_i) block pipeline to minimize program size
(per-call XLA/axon compile time scales with instruction count).

  - Self-loops folded into the edge list ON HOST (i->i edges appended);
    same d^-1/2 normalization makes them ordinary edges.
  - Nodes striped into 8 contiguous shards, padded to a multiple of 128.
  - Edges sharded by TARGET shard; per 128-target block, messages are
    gathered from a replicated node-major bf16 feature table (HBM) with
    dma_gather and segment-summed on the PE: psum[f, t] += M[e, f]^T S[e, t],
    one-hot S generated on-device (is_equal vs iota).
  - UNIFORM per-block chunk counts (CLu lo + CHu hi, padded with idx=0 /
    toff=-1) so the per-block body is loop-invariant -> tc.For_i_unrolled.
  - Per-layer table rebuilt via AllGather of local shards.
  - Readout: one-hot pool matmuls accumulated in SBUF, AllReduce, divide
    by counts, final matvec + sigmoid.
  - Payload: fp8(e4m3) pre-scaled x (bf16 table built on device), bf16
    weights, de-replicated int16 gather indices, int8 packed constants,
    deg^-1/2 broadcast table built on device.
"""

import hashlib

import ml_dtypes
import numpy as np

import concourse.bass as bass
import concourse.bacc as bacc
import concourse.mybir as mybir
from concourse.bass import ds
from concourse.tile import TileContext
from concourse.bass_utils import run_bass_kernel_spmd

F32 = mybir.dt.float32
BF16 = mybir.dt.bfloat16
F8 = mybir.dt.float8e4
I16 = mybir.dt.int16
I8 = mybir.dt.int8
OP = mybir.AluOpType
NCORES = 8
D = 128
G = 64  # number of graphs
SGRP = 8  # chunks per fused S-gen op
F8NP = mybir.dt.np(F8)
BF16NP = ml_dtypes.bfloat16


def cdiv(a, b):
    return -(-a // b)


def preprocess(edge_index, batch, N):
    """Host-side graph partitioning / index prep (numpy only, x-independent).

    Appends N self-edges (i->i) and lays out each target block's edges at
    a UNIFORM stride: block b owns chunks [b*CT, (b+1)*CT), the first CLu
    for table rows < LO, the next CHu for rows >= LO.
    """
    SHARD = cdiv(N, NCORES)
    SHARD_PAD = cdiv(SHARD, 128) * 128
    NB = SHARD_PAD // 128
    TBL = NCORES * SHARD_PAD
    LO = min(32768, TBL)

    row = np.concatenate([edge_index[0].astype(np.int64), np.arange(N)])
    col = np.concatenate([edge_index[1].astype(np.int64), np.arange(N)])
    deg = np.bincount(col, minlength=N).astype(np.float32)  # includes self
    dis = (1.0 / np.sqrt(deg)).astype(np.float32)

    srow = (row // SHARD) * SHARD_PAD + (row % SHARD)  # table row of source
    core = col // SHARD
    tloc = col % SHARD
    blk = tloc // 128
    toff = tloc % 128
    grp = (srow >= LO).astype(np.int64)

    counts = np.zeros((NCORES, NB, 2), np.int64)
    np.add.at(counts, (core, blk, grp), 1)
    CLu = int(cdiv(int(counts[:, :, 0].max()), 128))
    CHu = int(cdiv(int(counts[:, :, 1].max()), 128))
    CT = CLu + CHu
    TOT = NB * CT * 128

    IDX = np.zeros((NCORES, TOT), np.int64)
    TOF = np.full((NCORES, TOT), -1, np.int8)

    order = np.lexsort((grp, blk, core))
    c_s, b_s, g_s = core[order], blk[order], grp[order]
    s_s, t_s = srow[order], toff[order]
    key = (c_s * NB + b_s) * 2 + g_s
    starts = np.r_[0, np.flatnonzero(np.diff(key)) + 1]
    run_len = np.diff(np.r_[starts, len(key)])
    pos = np.arange(len(key)) - np.repeat(starts, run_len)
    dest = (b_s * CT + g_s * CLu) * 128 + pos
    IDX[c_s, dest] = s_s - g_s * LO
    TOF[c_s, dest] = t_s

    per_core = []
    for c in range(NCORES):
        lo_n, hi_n = c * SHARD, min((c + 1) * SHARD, N)
        n_real = hi_n - lo_n
        idx16 = np.ascontiguousarray(IDX[c].reshape(-1, 16).T.astype(np.int16))
        toff8 = np.ascontiguousarray(TOF[c].reshape(-1, 128).T)

        dis_sh = np.ones(SHARD_PAD, np.float32)
        dis_sh[:n_real] = dis[lo_n:hi_n]
        bat8 = np.full(SHARD_PAD, -1, np.int8)
        bat8[:n_real] = batch[lo_n:hi_n].astype(np.int8)
        per_core.append(dict(
            idx=idx16, toff8=toff8,
            dis_cols=np.ascontiguousarray(dis_sh.reshape(NB, 128).T),
            dis_flat=dis_sh,
            bat8=np.ascontiguousarray(bat8.reshape(NB, 128).T),
            lo_n=lo_n, hi_n=hi_n,
        ))

    gcounts = np.bincount(batch.astype(np.int64), minlength=G).astype(np.float32)
    recip = (1.0 / np.maximum(gcounts, 1.0)).astype(np.float32)
    meta = dict(N=N, SHARD=SHARD, SHARD_PAD=SHARD_PAD, NB=NB, TBL=TBL, LO=LO,
                CLu=CLu, CHu=CHu, TOT=TOT, recip=recip)
    return meta, per_core


def build_program(meta, iters=1, spkt=False, nq=4, msg_bufs=4, zt_bufs=2,
                  hwloop=True, unroll=4):
    NB, TBL, LO = meta["NB"], meta["TBL"], meta["LO"]
    SHARD_PAD = meta["SHARD_PAD"]
    CLu, CHu, TOT = meta["CLu"], meta["CHu"], meta["TOT"]
    CT = CLu + CHu
    NCH = TOT // 128
    W16 = TOT // 16
    TDT = BF16
    # packed-constant blob column offsets
    C8 = NCH + 128 + 128 + NB          # toff | iota | idn | bat
    O_IOTA, O_IDN, O_BAT = NCH, NCH + 128, NCH + 256
    CF = NB + 3 + 1 + 2                # dis | bcol | wf | aux
    O_DIS, O_BCOL = 0, NB
    O_WF, O_AUX = O_BCOL + 3, O_BCOL + 4

    nc = bacc.Bacc(None, target_bir_lowering=False, debug=False,
                   num_swdge_queues=nq)
    x_d = nc.declare_dram_parameter("x", [SHARD_PAD, D], F8, isOutput=False)
    idx_d = nc.declare_dram_parameter("idx", [16, W16], I16, isOutput=False)
    b8_d = nc.declare_dram_parameter("b8", [128, C8], I8, isOutput=False)
    cf_d = nc.declare_dram_parameter("cf", [128, CF], F32, isOutput=False)
    wb_d = nc.declare_dram_parameter("wb", [128, 384], BF16, isOutput=False)
    out_d = nc.declare_dram_parameter("out", [G, 1], F32, isOutput=True)

    rg = [list(range(NCORES))]

    with TileContext(nc) as tc:
        with (
            tc.tile_pool(name="const", bufs=1) as cp,
            tc.tile_pool(name="sb", bufs=3) as sb,
            tc.tile_pool(name="msg", bufs=msg_bufs) as mp,
            tc.tile_pool(name="spool", bufs=3) as spl,
            tc.tile_pool(name="ps", bufs=2, space="PSUM") as ps,
            tc.tile_pool(name="dram", bufs=1, space="DRAM") as dp,
        ):
            idx_t = cp.tile([128, W16], I16)
            b8_t = cp.tile([128, C8], I8)
            cf_t = cp.tile([128, CF], F32)
            wb_t = cp.tile([128, 384], BF16)
            toff_t = cp.tile([128, NCH], F32)
            iota_t = cp.tile([128, 128], F32)
            idn_t = cp.tile([128, 128], F32)
            bat_t = cp.tile([128, NB], F32)
            ones_t = cp.tile([128, 128], F32)
            disb_t = cp.tile([128, SHARD_PAD], F32)
            pp_sb = cp.tile([G, 128], F32)

            nc.sync.dma_start(out=b8_t[:], in_=b8_d[:])
            nc.sync.dma_start(out=cf_t[:], in_=cf_d[:])
            nc.sync.dma_start(out=wb_t[:], in_=wb_d[:])
            for k in range(8):
                nc.sync.dma_start(out=idx_t[16 * k:16 * (k + 1), :], in_=idx_d[:])
            nc.vector.tensor_copy(toff_t[:], b8_t[:, 0:NCH])
            nc.vector.tensor_copy(iota_t[:], b8_t[:, O_IOTA:O_IOTA + 128])
            nc.vector.tensor_copy(idn_t[:], b8_t[:, O_IDN:O_IDN + 128])
            nc.vector.tensor_copy(bat_t[:], b8_t[:, O_BAT:O_BAT + NB])
            nc.vector.memset(ones_t[:], 1.0)
            nc.vector.memset(pp_sb[:], 0.0)

            # deg^-1/2 broadcast table: disb[p, b*128+t] = dis[b*128+t]
            def disb_body(b):
                tmp = sb.tile([128, 128], F32, tag="yt")
                nc.vector.tensor_scalar_mul(tmp[:], ones_t[:],
                                            cf_t[:, ds(O_DIS + b, 1)])
                pst = ps.tile([128, 128], F32, tag="zt", bufs=zt_bufs)
                nc.tensor.transpose(pst[:], tmp[:], idn_t[:])
                nc.vector.tensor_copy(disb_t[:, ds(b * 128, 128)], pst[:])

            if hwloop:
                tc.For_i_unrolled(0, NB, 1, disb_body, max_unroll=unroll)
            else:
                for b in range(NB):
                    disb_body(b)

            for _it in range(iters):
                ag_in = [dp.tile([SHARD_PAD, D], TDT, tag=f"agin{i}_{_it}",
                                 name=f"agin{i}_{_it}") for i in range(3)]
                ag_out = [dp.tile([TBL, D], TDT, addr_space="Shared",
                                  tag=f"agout{i}_{_it}", name=f"agout{i}_{_it}")
                          for i in range(3)]
                ar_in = dp.tile([G, D], F32, tag=f"arin{_it}", name=f"arin{_it}")
                ar_out = dp.tile([G, D], F32, addr_space="Shared",
                                 tag=f"arout{_it}", name=f"arout{_it}")

                # ---- table 0 = x * dis (fp8 upload -> bf16) + AllGather ----
                def xcv_body(b):
                    xf8 = sb.tile([128, 128], F8, tag="xf8")
                    nc.sync.dma_start(out=xf8[:], in_=x_d[ds(b * 128, 128), :])
                    xcb = sb.tile([128, 128], TDT, tag="tblblk")
                    nc.vector.tensor_copy(xcb[:], xf8[:])
                    nc.sync.dma_start(out=ag_in[0][ds(b * 128, 128), :],
                                      in_=xcb[:])

                if hwloop:
                    tc.For_i_unrolled(0, NB, 1, xcv_body, max_unroll=unroll)
                else:
                    for b in range(NB):
                        xcv_body(b)
                nc.gpsimd.collective_compute(
                    "AllGather", OP.bypass, replica_groups=rg,
                    ins=[ag_in[0].opt()], outs=[ag_out[0].opt()])

                # ---- 3 GCN layers ----
                for li in range(3):
                    last = li == 2
                    tbl_dram = ag_out[li]

                    def blk_body(b, li=li, last=last, tbl_dram=tbl_dram):
                        mlo = mp.tile([128, CLu, 128], TDT, tag="mlo")
                        nc.gpsimd.dma_gather(
                            mlo[:, :, :], tbl_dram[0:LO, :],
                            idx_t[:, ds(b * (CT * 8), CLu * 8)],
                            CLu * 128, CLu * 128, D, single_packet=bool(spkt),
                            queue_num=(2 * li) % nq)
                        mhi = mp.tile([128, CHu, 128], TDT, tag="mhi")
                        nc.gpsimd.dma_gather(
                            mhi[:, :, :], tbl_dram[LO:TBL, :],
                            idx_t[:, ds(b * (CT * 8) + CLu * 8, CHu * 8)],
                            CHu * 128, CHu * 128, D, single_packet=bool(spkt),
                            queue_num=(2 * li + 1) % nq)
                        zt = ps.tile([128, 128], F32, tag="zt", bufs=zt_bufs)
                        k = 0
                        for mt, g0, cnt in ((mlo, 0, CLu), (mhi, CLu, CHu)):
                            for c00 in range(0, cnt, SGRP):
                                gn = min(SGRP, cnt - c00)
                                s8 = spl.tile([128, SGRP, 128], TDT, tag="s8")
                                nc.vector.tensor_tensor(
                                    s8[:, :gn, :],
                                    iota_t[:].unsqueeze(1).broadcast_to(
                                        (128, gn, 128)),
                                    toff_t[:, ds(b * CT + g0 + c00, gn)]
                                    .unsqueeze(2).broadcast_to((128, gn, 128)),
                                    OP.is_equal)
                                for c in range(gn):
                                    nc.tensor.matmul(
                                        zt[:], mt[:, c00 + c, :], s8[:, c, :],
                                        start=(k == 0), stop=(k == CT - 1))
                                    k += 1
                        # epilogue (transposed): yT = zT*dis ; ht = W @ yT ;
                        # xT = relu(ht + b) ; xp = xT^T ; table = xp * dis
                        yt = sb.tile([128, 128], TDT, tag="yt")
                        nc.vector.tensor_mul(
                            yt[:], zt[:], disb_t[:, ds(b * 128, 128)])
                        ht = ps.tile([128, 128], F32, tag="ht")
                        nc.tensor.matmul(ht[:], wb_t[:, li * 128:(li + 1) * 128],
                                         yt[:], start=True, stop=True)
                        xt = sb.tile([128, 128], F32, tag="xt")
                        nc.scalar.activation(
                            xt[:], ht[:], mybir.ActivationFunctionType.Relu,
                            bias=cf_t[:, O_BCOL + li:O_BCOL + li + 1])
                        xp = ps.tile([128, 128], F32, tag="xp")
                        nc.tensor.transpose(xp[:], xt[:], idn_t[:])
                        if not last:
                            tb = sb.tile([128, 128], TDT, tag="tblblk")
                            nc.vector.tensor_scalar_mul(
                                tb[:], xp[:], cf_t[:, ds(O_DIS + b, 1)])
                            nc.sync.dma_start(
                                out=ag_in[li + 1][ds(b * 128, 128), :],
                                in_=tb[:])
                        else:
                            xs = sb.tile([128, 128], F32, tag="xs")
                            nc.vector.tensor_copy(xs[:], xp[:])
                            sp = spl.tile([128, G], F32, tag="sp", bufs=2)
                            nc.vector.tensor_scalar(
                                sp[:], iota_t[:, :G], bat_t[:, ds(b, 1)], None,
                                OP.is_equal)
                            ppp = ps.tile([G, 128], F32, tag="ht")
                            nc.tensor.matmul(ppp[:], sp[:], xs[:],
                                             start=True, stop=True)
                            nc.vector.tensor_tensor(pp_sb[:], pp_sb[:], ppp[:],
                                                    OP.add)

                    if hwloop:
                        tc.For_i_unrolled(0, NB, 1, blk_body, max_unroll=unroll)
                    else:
                        for b in range(NB):
                            blk_body(b)
                    if not last:
                        nc.gpsimd.collective_compute(
                            "AllGather", OP.bypass, replica_groups=rg,
                            ins=[ag_in[li + 1].opt()],
                            outs=[ag_out[li + 1].opt()])

                # ---- readout ----
                nc.sync.dma_start(out=ar_in[:], in_=pp_sb[:])
                nc.gpsimd.collective_compute(
                    "AllReduce", OP.add, replica_groups=rg,
                    ins=[ar_in.opt()], outs=[ar_out.opt()])
                p2 = sb.tile([G, 128], F32, tag="p2")
                nc.sync.dma_start(out=p2[:], in_=ar_out[:])
                nc.vector.tensor_scalar_mul(p2[:], p2[:],
                                            cf_t[0:G, O_AUX:O_AUX + 1])
                pt = ps.tile([128, G], F32, tag="zt")
                nc.tensor.transpose(pt[:], p2[:], idn_t[:G, :G])
                pts = sb.tile([128, G], F32, tag="pts")
                nc.vector.tensor_copy(pts[:], pt[:])
                fin = ps.tile([G, 1], F32, tag="ht")
                nc.tensor.matmul(fin[:], pts[:], cf_t[:, O_WF:O_WF + 1],
                                 start=True, stop=True)
                ob = sb.tile([G, 1], F32, tag="ob")
                nc.scalar.activation(ob[:], fin[:],
                                     mybir.ActivationFunctionType.Sigmoid,
                                     bias=cf_t[0:G, O_AUX + 1:O_AUX + 2])
                nc.sync.dma_start(out=out_d[:], in_=ob[:])

    nc.compile()
    return nc


def make_in_maps(meta, per_core, x, W1, b1, W2, b2, W3, b3, Wf, bf):
    NB, SHARD_PAD = meta["NB"], meta["SHARD_PAD"]
    iota8 = np.broadcast_to(np.arange(128, dtype=np.int8), (128, 128))
    idn8 = np.eye(128, dtype=np.int8)
    wcat = np.concatenate([W1, W2, W3], axis=1).astype(BF16NP)  # [128, 384]
    bcol = np.stack([b1, b2, b3], axis=1).astype(np.float32)    # [128, 3]
    wfc = np.asarray(Wf, np.float32).reshape(128, 1)
    auxp = np.zeros((128, 2), np.float32)
    auxp[:G, 0] = meta["recip"]
    auxp[:G, 1] = float(np.asarray(bf).reshape(-1)[0])
    in_maps = []
    for c in range(NCORES):
        pc = per_core[c]
        n_real = pc["hi_n"] - pc["lo_n"]
        x_sh = np.zeros((SHARD_PAD, D), np.float32)
        x_sh[:n_real] = x[pc["lo_n"]:pc["hi_n"]]
        t0 = (x_sh * pc["dis_flat"][:, None]).astype(F8NP)
        b8 = np.concatenate(
            [pc["toff8"], iota8, idn8, pc["bat8"]], axis=1).astype(np.int8)
        cf = np.concatenate(
            [pc["dis_cols"], bcol, wfc, auxp], axis=1).astype(np.float32)
        in_maps.append(dict(x=t0, idx=pc["idx"], b8=b8, cf=cf, wb=wcat))
    return in_maps


_CACHE = {}


def kernel(x, edge_index, batch, W1, b1, W2, b2, W3, b3, Wf, bf):
    x = np.asarray(x, np.float32)
    edge_index = np.asarray(edge_index)
    batch = np.asarray(batch)
    key = hashlib.md5(
        edge_index.tobytes() + batch.tobytes() + str(x.shape).encode()
    ).hexdigest()
    ent = _CACHE.get(key)
    if ent is None:
        meta, per_core = preprocess(edge_index, batch, x.shape[0])
        nc = build_program(meta)
        _CACHE[key] = ent = (meta, per_core, nc)
    meta, per_core, nc = ent
    in_maps = make_in_maps(meta, per_core, x, W1, b1, W2, b2, W3, b3, Wf, bf)
    res = run_bass_kernel_spmd(nc, in_maps, list(range(NCORES)))
    return np.asarray(res.results[0]["out"], np.float32)
